# revision 4
# baseline (speedup 1.0000x reference)
"""HGNN (DGL-style hypergraph conv x3) Bass kernel for trn2, 8 NeuronCores.

Math (per layer, weights/bias W,b):
    out = (D_v^-1 B^T D_e^-1 B X) @ W + b         (+ relu / final log_softmax)
where B is the (edge x node) incidence matrix given by (node_idx, edge_idx)
pairs. W commutes past the (linear, row-wise-scaled) aggregations, so each
layer does: gather+segment-sum into edges, normalize, all-gather, gather+
segment-sum into nodes, normalize, then a small dense matmul with W.

Sharding: edges / nodes are 1-D range-partitioned across the 8 cores; the
incidence nnz are assigned to the core owning the edge (edge-side pass) /
the node (node-side pass). Feature tables (X, per-layer node features, edge
aggregates) are replicated (bf16) via AllGather so row gathers are always
local.

Row gathers use gpsimd.dma_gather (int16 indices, wrapped in 16 partitions
and replicated across the 8 Q7 cores). Indices are capped at 32767, so the
50000-row node tables are gathered in two sweeps (rows < 25000 and >=
25000, table view sliced accordingly); per-edge-block partial sums from the
two sweeps are combined in an SBUF accumulator before normalization. The
20000-row edge table needs a single sweep.

Segment sums run on the tensor engine in bf16: for each 128-nnz tile of the
sorted incidence stream, a 0/1 selection matrix S^T (built on the vector
engine by comparing per-nnz local segment ids against an iota row) maps
gathered rows into a fp32 PSUM accumulator indexed by segment within a
128-wide block. Padding slots carry segment id -1 (gather row 0, weight 0).
"""
import hashlib
import sys

import numpy as np

sys.path.insert(0, "/opt/trn_rl_repo")

import ml_dtypes

V, E, NNZ = 50000, 20000, 500000
D = 256
F_OUT = [256, 256, 40]
NCORES = 8
EPC = E // NCORES          # 2500 edges per core
VPC = V // NCORES          # 6250 nodes per core
NBE = (EPC + 127) // 128   # 20 edge blocks per core
NBV = (VPC + 127) // 128   # 49 node blocks per core
TC = 8                     # 128-row tiles per dma_gather chunk (1024 idx)
VHALF = V // 2

P = 128


def _wrap_idx16(flat):
    """dma_gather index layout: int16, index i at [i % 16, i // 16],
    replicated across the 8 Q7 core partition groups -> [128, n/16]."""
    assert flat.size % 16 == 0
    w = flat.astype(np.int16).reshape(-1, 16).T.copy()
    return np.tile(w, (8, 1))


def _side_arrays(seg_local, other_idx, n_blocks, TB):
    """Build gather-index / local-segment-id arrays for one core's sorted
    nnz stream (sorted by seg_local). TB[b] = padded tile count for block b
    (common across cores). Returns (idx_flat [ntiles*128], luc [128, ntiles])
    where flat order within a tile chunk is i = t*128 + p."""
    n_tiles = sum(TB)
    idx = np.zeros((n_tiles, P), dtype=np.int64)     # [tile, partition]
    luc = np.full((P, n_tiles), -1.0, dtype=np.float32)
    counts = np.bincount(np.asarray(seg_local) // P, minlength=n_blocks)
    offs = np.concatenate([[0], np.cumsum(counts)])
    col = 0
    for b in range(n_blocks):
        lo, hi = offs[b], offs[b + 1]
        s = np.arange(hi - lo)
        t, p = s // P, s % P
        idx[col + t, p] = other_idx[lo:hi]
        luc[p, col + t] = (seg_local[lo:hi] - P * b).astype(np.float32)
        col += TB[b]
    return idx.reshape(-1), luc


def _pad_tb(TB):
    TB = list(TB)
    TB[-1] += (-sum(TB)) % TC
    return TB


def _preprocess(node_idx, edge_idx):
    ni = np.asarray(node_idx, dtype=np.int64)
    ei = np.asarray(edge_idx, dtype=np.int64)
    deg_e = np.bincount(ei, minlength=E)
    deg_v = np.bincount(ni, minlength=V)
    rde_full = (1.0 / np.maximum(deg_e, 1)).astype(np.float32)
    rdv_full = (1.0 / np.maximum(deg_v, 1)).astype(np.float32)

    # ---- edge-side: nnz grouped by owning edge range, split by node half
    # (dma_gather int16 indices cap the table at 32768 rows), sorted by edge
    e_sorted = {}   # (core, half) -> (edge_local, node_local_in_half)
    TBe = {0: [0] * NBE, 1: [0] * NBE}
    for c in range(NCORES):
        sel_c = (ei >= c * EPC) & (ei < (c + 1) * EPC)
        for h in range(2):
            sel = sel_c & ((ni >= VHALF) if h else (ni < VHALF))
            el = ei[sel] - c * EPC
            nn = ni[sel] - h * VHALF
            order = np.argsort(el, kind="stable")
            e_sorted[(c, h)] = (el[order], nn[order])
            cnt = np.bincount(el // P, minlength=NBE)
            for b in range(NBE):
                TBe[h][b] = max(TBe[h][b], -(-int(cnt[b]) // P))
    for h in range(2):
        TBe[h] = _pad_tb(TBe[h])
    TEl, TEh = sum(TBe[0]), sum(TBe[1])

    # ---- node-side: nnz grouped by owning node range, sorted by node
    v_sorted = []
    TBv = [0] * NBV
    for c in range(NCORES):
        sel = (ni >= c * VPC) & (ni < (c + 1) * VPC)
        vl = ni[sel] - c * VPC
        ee = ei[sel]
        order = np.argsort(vl, kind="stable")
        v_sorted.append((vl[order], ee[order]))
        cnt = np.bincount(vl // P, minlength=NBV)
        for b in range(NBV):
            TBv[b] = max(TBv[b], -(-int(cnt[b]) // P))
    TBv = _pad_tb(TBv)
    TV = sum(TBv)

    per_core = []
    for c in range(NCORES):
        idxel, lucel = _side_arrays(*e_sorted[(c, 0)], NBE, TBe[0])
        idxeh, luceh = _side_arrays(*e_sorted[(c, 1)], NBE, TBe[1])
        idxv, lucv = _side_arrays(*v_sorted[c], NBV, TBv)
        rde = np.ones((P, NBE), dtype=np.float32)
        for b in range(NBE):
            n = min(P, EPC - P * b)
            rde[:n, b] = rde_full[c * EPC + P * b: c * EPC + P * b + n]
        rdv = np.ones((P, NBV), dtype=np.float32)
        for b in range(NBV):
            n = min(P, VPC - P * b)
            rdv[:n, b] = rdv_full[c * VPC + P * b: c * VPC + P * b + n]
        per_core.append(dict(
            idxel=_wrap_idx16(idxel), lucel=lucel,
            idxeh=_wrap_idx16(idxeh), luceh=luceh,
            idxv=_wrap_idx16(idxv), lucv=lucv,
            rde=rde, rdv=rdv))
    return dict(TBel=TBe[0], TBeh=TBe[1], TBv=TBv,
                TEl=TEl, TEh=TEh, TV=TV, per_core=per_core)


def _flatten_blocks(TB):
    """[(block, is_first, is_last)] per tile."""
    out = []
    for b, T in enumerate(TB):
        for t in range(T):
            out.append((b, t == 0, t == T - 1))
    return out


def _build(meta, debug=None):
    import concourse.bacc as bacc
    import concourse.bass as bass
    import concourse.mybir as mybir
    import concourse.tile as tile

    f32 = mybir.dt.float32
    bf16 = mybir.dt.bfloat16
    i16 = mybir.dt.int16
    TEl, TEh, TV = meta["TEl"], meta["TEh"], meta["TV"]
    tiles_el = _flatten_blocks(meta["TBel"])
    tiles_eh = _flatten_blocks(meta["TBeh"])
    tiles_v = _flatten_blocks(meta["TBv"])

    nc = bacc.Bacc("TRN2", target_bir_lowering=False, debug=False,
                   num_devices=NCORES)

    xt = nc.dram_tensor("xt", [V, D], bf16, kind="ExternalInput")
    idxel_d = nc.dram_tensor("idxel", [P, TEl * 8], i16, kind="ExternalInput")
    lucel_d = nc.dram_tensor("lucel", [P, TEl], f32, kind="ExternalInput")
    idxeh_d = nc.dram_tensor("idxeh", [P, TEh * 8], i16, kind="ExternalInput")
    luceh_d = nc.dram_tensor("luceh", [P, TEh], f32, kind="ExternalInput")
    idxv_d = nc.dram_tensor("idxv", [P, TV * 8], i16, kind="ExternalInput")
    lucv_d = nc.dram_tensor("lucv", [P, TV], f32, kind="ExternalInput")
    rde_d = nc.dram_tensor("rde", [P, NBE], f32, kind="ExternalInput")
    rdv_d = nc.dram_tensor("rdv", [P, NBV], f32, kind="ExternalInput")
    w_d = [nc.dram_tensor(f"w{i+1}", [D, F_OUT[i]], bf16, kind="ExternalInput")
           for i in range(3)]
    b_d = [nc.dram_tensor(f"b{i+1}x", [P, F_OUT[i]], f32, kind="ExternalInput")
           for i in range(3)]
    iota_d = nc.dram_tensor("iota", [P, P], f32, kind="ExternalInput")
    ident_d = nc.dram_tensor("ident", [P, P], bf16, kind="ExternalInput")
    if debug == "e0":
        out_d = nc.dram_tensor("out", [EPC, D], f32, kind="ExternalOutput")
    elif debug in ("v0", "v1"):
        out_d = nc.dram_tensor("out", [VPC, D], f32, kind="ExternalOutput")
    else:
        out_d = nc.dram_tensor("out", [VPC, F_OUT[2]], f32, kind="ExternalOutput")

    eloc = [nc.dram_tensor(f"eloc{i}", [EPC, D], bf16) for i in range(2)]
    etab = [nc.dram_tensor(f"etab{i}", [E, D], bf16) for i in range(2)]
    vloc = [nc.dram_tensor(f"vloc{i}", [VPC, D], bf16) for i in range(2)]
    vtab = [nc.dram_tensor(f"vtab{i}", [V, D], bf16) for i in range(2)]
    groups = [list(range(NCORES))]

    with tile.TileContext(nc) as tc:
        with (
            tc.tile_pool(name="const", bufs=1) as cpool,
            tc.tile_pool(name="g", bufs=6) as gpool,
            tc.tile_pool(name="st", bufs=4) as spool,
            tc.tile_pool(name="eo", bufs=3) as eopool,
            tc.tile_pool(name="va", bufs=2) as vapool,
            tc.tile_pool(name="at", bufs=2) as atpool,
            tc.tile_pool(name="ob", bufs=3) as obpool,
            tc.tile_pool(name="sm", bufs=2) as smpool,
            tc.tile_pool(name="ps", bufs=3, space="PSUM") as pspool,
            tc.tile_pool(name="pt", bufs=2, space="PSUM") as ptpool,
            tc.tile_pool(name="po", bufs=2, space="PSUM") as popool,
        ):
            def load_const(dram, shape, tag, dtype=f32):
                t = cpool.tile(shape, dtype, tag=tag)
                nc.sync.dma_start(out=t[:], in_=dram[:])
                return t

            idxel_sb = load_const(idxel_d, [P, TEl * 8], "idxel", i16)
            lucel_sb = load_const(lucel_d, [P, TEl], "lucel")
            idxeh_sb = load_const(idxeh_d, [P, TEh * 8], "idxeh", i16)
            luceh_sb = load_const(luceh_d, [P, TEh], "luceh")
            idxv_sb = load_const(idxv_d, [P, TV * 8], "idxv", i16)
            lucv_sb = load_const(lucv_d, [P, TV], "lucv")
            rde_sb = load_const(rde_d, [P, NBE], "rde")
            rdv_sb = load_const(rdv_d, [P, NBV], "rdv")
            iota_sb = load_const(iota_d, [P, P], "iota")
            ident_sb = load_const(ident_d, [P, P], "ident", bf16)
            w_sb = []
            for i in range(3):
                t0 = cpool.tile([P, F_OUT[i]], bf16, tag=f"w{i}a")
                t1 = cpool.tile([P, F_OUT[i]], bf16, tag=f"w{i}b")
                nc.sync.dma_start(out=t0[:], in_=w_d[i][0:P, :])
                nc.sync.dma_start(out=t1[:], in_=w_d[i][P:2 * P, :])
                w_sb.append((t0, t1))
            b_sb = [load_const(b_d[i], [P, F_OUT[i]], f"bias{i}")
                    for i in range(3)]
            # SBUF accumulator for the two-sweep edge pass
            eacc = cpool.tile([P, NBE * D], f32, tag="eacc")

            def segsum(table_ap, idx_sb, luc_sb, tiles, n_tiles, on_done):
                """Sorted-stream segment sum: gather TC tiles per dma_gather
                chunk, select with is_equal one-hots, accumulate per block
                in PSUM; on_done(block, psum) at each block's last tile."""
                psums = {}
                for ch in range(n_tiles // TC):
                    g = gpool.tile([P, TC * D], bf16, tag="g")
                    nc.gpsimd.dma_gather(
                        g[:].rearrange("p (t d) -> p t d", d=D),
                        table_ap,
                        idx_sb[:, ch * TC * 8:(ch + 1) * TC * 8],
                        TC * P, TC * P, D)
                    st = spool.tile([P, TC * P], bf16, tag="st")
                    nc.vector.tensor_tensor(
                        out=st[:].rearrange("p (t i) -> p t i", i=P),
                        in0=luc_sb[:, ch * TC:(ch + 1) * TC]
                            .unsqueeze(2).to_broadcast([P, TC, P]),
                        in1=iota_sb[:].unsqueeze(1).to_broadcast([P, TC, P]),
                        op=mybir.AluOpType.is_equal,
                    )
                    for j in range(TC):
                        b, first, last = tiles[ch * TC + j]
                        if first:
                            psums[b] = pspool.tile([P, D], f32, tag="ps",
                                                   name=f"ps{b}")
                        nc.tensor.matmul(
                            out=psums[b][:],
                            lhsT=st[:, j * P:(j + 1) * P],
                            rhs=g[:, j * D:(j + 1) * D],
                            start=first, stop=last,
                        )
                        if last:
                            on_done(b, psums.pop(b))

            for layer in range(3):
                table_in = xt if layer == 0 else vtab[(layer + 1) % 2]
                Fo = F_OUT[layer]
                dump_e = debug == "e0" and layer == 0
                dump_v = debug == f"v{layer}"

                # --- edge side: two sweeps (node halves), SBUF-accumulated
                def e_lo(b, ps):
                    nc.vector.tensor_copy(eacc[:, b * D:(b + 1) * D], ps[:])

                def e_hi(b, ps, layer=layer, dump_e=dump_e):
                    esb = eopool.tile([P, D], bf16, tag="eo")
                    nc.vector.tensor_add(out=eacc[:, b * D:(b + 1) * D],
                                         in0=eacc[:, b * D:(b + 1) * D],
                                         in1=ps[:])
                    nc.vector.tensor_scalar_mul(
                        esb[:], eacc[:, b * D:(b + 1) * D], rde_sb[:, b:b + 1])
                    cnt = min(P, EPC - P * b)
                    if dump_e:
                        esf = eopool.tile([P, D], f32, tag="eof")
                        nc.vector.tensor_scalar_mul(
                            esf[:], eacc[:, b * D:(b + 1) * D],
                            rde_sb[:, b:b + 1])
                        nc.sync.dma_start(out=out_d[P * b:P * b + cnt, :],
                                          in_=esf[:cnt, :])
                    else:
                        nc.sync.dma_start(
                            out=eloc[layer % 2][P * b:P * b + cnt, :],
                            in_=esb[:cnt, :])

                segsum(xt[0:VHALF, :] if layer == 0
                       else vtab[(layer + 1) % 2][0:VHALF, :],
                       idxel_sb, lucel_sb, tiles_el, TEl, e_lo)
                segsum(xt[VHALF:V, :] if layer == 0
                       else vtab[(layer + 1) % 2][VHALF:V, :],
                       idxeh_sb, luceh_sb, tiles_eh, TEh, e_hi)
                if dump_e:
                    break
                nc.gpsimd.collective_compute(
                    "AllGather", mybir.AluOpType.bypass, replica_groups=groups,
                    ins=[eloc[layer % 2][:].opt()], outs=[etab[layer % 2][:].opt()],
                )

                def v_done(b, ps, layer=layer, Fo=Fo, dump_v=dump_v):
                    asb = vapool.tile([P, D], bf16, tag="va")
                    nc.vector.tensor_scalar_mul(asb[:], ps[:], rdv_sb[:, b:b + 1])
                    ptp = ptpool.tile([P, D], bf16, tag="pt")
                    nc.tensor.transpose(out=ptp[:, 0:P], in_=asb[:, 0:P],
                                        identity=ident_sb[:])
                    nc.tensor.transpose(out=ptp[:, P:D], in_=asb[:, P:D],
                                        identity=ident_sb[:])
                    att = atpool.tile([P, D], bf16, tag="at")
                    nc.vector.tensor_copy(att[:], ptp[:])
                    pop = popool.tile([P, Fo], f32, tag="po")
                    nc.tensor.matmul(out=pop[:], lhsT=att[:, 0:P],
                                     rhs=w_sb[layer][0][:], start=True, stop=False)
                    nc.tensor.matmul(out=pop[:], lhsT=att[:, P:D],
                                     rhs=w_sb[layer][1][:], start=False, stop=True)
                    cnt = min(P, VPC - P * b)
                    if layer < 2:
                        osb = obpool.tile([P, Fo], bf16, tag="ob")
                        nc.vector.tensor_add(out=osb[:], in0=pop[:],
                                             in1=b_sb[layer][:])
                        nc.scalar.activation(out=osb[:], in_=osb[:],
                                             func=mybir.ActivationFunctionType.Relu)
                        if dump_v:
                            osf = obpool.tile([P, Fo], f32, tag="obf")
                            nc.vector.tensor_copy(osf[:], osb[:])
                            nc.sync.dma_start(out=out_d[P * b:P * b + cnt, :],
                                              in_=osf[:cnt, :])
                        else:
                            nc.sync.dma_start(
                                out=vloc[layer % 2][P * b:P * b + cnt, :],
                                in_=osb[:cnt, :])
                    else:
                        osb = obpool.tile([P, Fo], f32, tag="obf32")
                        nc.vector.tensor_add(out=osb[:], in0=pop[:],
                                             in1=b_sb[layer][:])
                        negmax = smpool.tile([P, 1], f32, tag="negmax")
                        nc.vector.tensor_reduce(
                            out=negmax[:], in_=osb[:], axis=mybir.AxisListType.X,
                            op=mybir.AluOpType.max, negate=True)
                        expt = smpool.tile([P, Fo], f32, tag="expt")
                        sumexp = smpool.tile([P, 1], f32, tag="sumexp")
                        nc.scalar.activation(
                            out=expt[:], in_=osb[:],
                            func=mybir.ActivationFunctionType.Exp,
                            bias=negmax[:, 0:1], accum_out=sumexp[:, 0:1])
                        logsum = smpool.tile([P, 1], f32, tag="logsum")
                        nc.scalar.activation(
                            out=logsum[:], in_=sumexp[:],
                            func=mybir.ActivationFunctionType.Ln)
                        shift = smpool.tile([P, 1], f32, tag="shift")
                        nc.vector.tensor_sub(out=shift[:], in0=negmax[:],
                                             in1=logsum[:])
                        res = smpool.tile([P, Fo], f32, tag="res")
                        nc.vector.tensor_scalar_add(res[:], osb[:], shift[:, 0:1])
                        nc.sync.dma_start(out=out_d[P * b:P * b + cnt, :],
                                          in_=res[:cnt, :])

                segsum(etab[layer % 2][:], idxv_sb, lucv_sb, tiles_v, TV, v_done)
                if dump_v:
                    break
                if layer < 2:
                    nc.gpsimd.collective_compute(
                        "AllGather", mybir.AluOpType.bypass,
                        replica_groups=groups,
                        ins=[vloc[layer % 2][:].opt()],
                        outs=[vtab[layer % 2][:].opt()],
                    )
    nc.finalize()
    return nc


_CACHE = {}


def make_in_maps(meta, X, W1, b1, W2, b2, W3, b3):
    iota = np.broadcast_to(np.arange(P, dtype=np.float32), (P, P)).copy()
    ident = np.eye(P, dtype=np.float32).astype(ml_dtypes.bfloat16)
    Xb = np.ascontiguousarray(np.asarray(X, dtype=np.float32)).astype(
        ml_dtypes.bfloat16)
    ws = [np.ascontiguousarray(np.asarray(w, dtype=np.float32)).astype(
        ml_dtypes.bfloat16) for w in (W1, W2, W3)]
    bs = [np.broadcast_to(np.asarray(b, dtype=np.float32), (P, len(b))).copy()
          for b in (b1, b2, b3)]
    in_maps = []
    for c in range(NCORES):
        pc = meta["per_core"][c]
        in_maps.append({
            "xt": Xb,
            "idxel": pc["idxel"], "lucel": pc["lucel"],
            "idxeh": pc["idxeh"], "luceh": pc["luceh"],
            "idxv": pc["idxv"], "lucv": pc["lucv"],
            "rde": pc["rde"], "rdv": pc["rdv"],
            "w1": ws[0], "w2": ws[1], "w3": ws[2],
            "b1x": bs[0], "b2x": bs[1], "b3x": bs[2],
            "iota": iota, "ident": ident,
        })
    return in_maps


def kernel(X, node_idx, edge_idx, W1, b1, W2, b2, W3, b3):
    from concourse import bass_utils

    ni = np.asarray(node_idx, dtype=np.int32)
    ei = np.asarray(edge_idx, dtype=np.int32)

    key = hashlib.sha1(ni.tobytes() + ei.tobytes()).hexdigest()
    if key not in _CACHE:
        meta = _preprocess(ni, ei)
        nc = _build(meta)
        _CACHE[key] = (meta, nc)
    meta, nc = _CACHE[key]

    in_maps = make_in_maps(meta, X, W1, b1, W2, b2, W3, b3)
    res = bass_utils.run_bass_kernel_spmd(nc, in_maps, list(range(NCORES)))
    return np.concatenate([res.results[c]["out"] for c in range(NCORES)], axis=0)


# revision 5
# speedup vs baseline: 1.0533x; 1.0533x over previous
"""HGNN (DGL-style hypergraph conv x3) Bass kernel for trn2, 8 NeuronCores.

Math (per layer, weights/bias W,b):
    out = (D_v^-1 B^T D_e^-1 B X) @ W + b         (+ relu / final log_softmax)
where B is the (edge x node) incidence matrix given by (node_idx, edge_idx)
pairs. W commutes past the (linear, row-wise-scaled) aggregations, so each
layer does: gather+segment-sum into edges, normalize, all-gather, gather+
segment-sum into nodes, normalize, then a small dense matmul with W.

Sharding: edges / nodes are 1-D range-partitioned across the 8 cores; the
incidence nnz are assigned to the core owning the edge (edge-side pass) /
the node (node-side pass). Feature tables (X, per-layer node features, edge
aggregates) are replicated (bf16) via AllGather so row gathers are always
local.

Row gathers use gpsimd.dma_gather (int16 indices, wrapped in 16 partitions
and replicated across the 8 Q7 cores). Indices are capped at 32767, so the
50000-row node tables are gathered in two sweeps (rows < 25000 and >=
25000, table view sliced accordingly); per-edge-block partial sums from the
two sweeps are combined in an SBUF accumulator before normalization. The
20000-row edge table needs a single sweep.

Segment sums run on the tensor engine in bf16: for each 128-nnz tile of the
sorted incidence stream, a 0/1 selection matrix S^T (built on the vector
engine by comparing per-nnz local segment ids against an iota row) maps
gathered rows into a fp32 PSUM accumulator indexed by segment within a
128-wide block. Padding slots carry segment id -1 (gather row 0, weight 0).
"""
import hashlib
import sys

import numpy as np

sys.path.insert(0, "/opt/trn_rl_repo")

import ml_dtypes

V, E, NNZ = 50000, 20000, 500000
D = 256
F_OUT = [256, 256, 40]
NCORES = 8
EPC = E // NCORES          # 2500 edges per core
VPC = V // NCORES          # 6250 nodes per core
NBE = (EPC + 127) // 128   # 20 edge blocks per core
NBV = (VPC + 127) // 128   # 49 node blocks per core
TC = 8                     # 128-row tiles per dma_gather chunk (1024 idx)
VHALF = V // 2

P = 128


def _wrap_idx16(flat):
    """dma_gather index layout: int16, index i at [i % 16, i // 16],
    replicated across the 8 Q7 core partition groups -> [128, n/16]."""
    assert flat.size % 16 == 0
    w = flat.astype(np.int16).reshape(-1, 16).T.copy()
    return np.tile(w, (8, 1))


def _side_arrays(seg_local, other_idx, n_blocks, TB):
    """Build gather-index / local-segment-id arrays for one core's sorted
    nnz stream (sorted by seg_local). TB[b] = padded tile count for block b
    (common across cores). Returns (idx_flat [ntiles*128], luc [128, ntiles])
    where flat order within a tile chunk is i = t*128 + p."""
    n_tiles = sum(TB)
    idx = np.zeros((n_tiles, P), dtype=np.int64)     # [tile, partition]
    luc = np.full((P, n_tiles), -1.0, dtype=np.float32)
    counts = np.bincount(np.asarray(seg_local) // P, minlength=n_blocks)
    offs = np.concatenate([[0], np.cumsum(counts)])
    col = 0
    for b in range(n_blocks):
        lo, hi = offs[b], offs[b + 1]
        s = np.arange(hi - lo)
        t, p = s // P, s % P
        idx[col + t, p] = other_idx[lo:hi]
        luc[p, col + t] = (seg_local[lo:hi] - P * b).astype(np.float32)
        col += TB[b]
    return idx.reshape(-1), luc


def _pad_tb(TB):
    TB = list(TB)
    TB[-1] += (-sum(TB)) % TC
    return TB


def _preprocess(node_idx, edge_idx):
    ni = np.asarray(node_idx, dtype=np.int64)
    ei = np.asarray(edge_idx, dtype=np.int64)
    deg_e = np.bincount(ei, minlength=E)
    deg_v = np.bincount(ni, minlength=V)
    rde_full = (1.0 / np.maximum(deg_e, 1)).astype(np.float32)
    rdv_full = (1.0 / np.maximum(deg_v, 1)).astype(np.float32)

    # ---- edge-side: nnz grouped by owning edge range, split by node half
    # (dma_gather int16 indices cap the table at 32768 rows), sorted by edge
    e_sorted = {}   # (core, half) -> (edge_local, node_local_in_half)
    TBe = {0: [0] * NBE, 1: [0] * NBE}
    for c in range(NCORES):
        sel_c = (ei >= c * EPC) & (ei < (c + 1) * EPC)
        for h in range(2):
            sel = sel_c & ((ni >= VHALF) if h else (ni < VHALF))
            el = ei[sel] - c * EPC
            nn = ni[sel] - h * VHALF
            order = np.argsort(el, kind="stable")
            e_sorted[(c, h)] = (el[order], nn[order])
            cnt = np.bincount(el // P, minlength=NBE)
            for b in range(NBE):
                TBe[h][b] = max(TBe[h][b], -(-int(cnt[b]) // P))
    for h in range(2):
        TBe[h] = _pad_tb(TBe[h])
    TEl, TEh = sum(TBe[0]), sum(TBe[1])

    # ---- node-side: nnz grouped by owning node range, sorted by node
    v_sorted = []
    TBv = [0] * NBV
    for c in range(NCORES):
        sel = (ni >= c * VPC) & (ni < (c + 1) * VPC)
        vl = ni[sel] - c * VPC
        ee = ei[sel]
        order = np.argsort(vl, kind="stable")
        v_sorted.append((vl[order], ee[order]))
        cnt = np.bincount(vl // P, minlength=NBV)
        for b in range(NBV):
            TBv[b] = max(TBv[b], -(-int(cnt[b]) // P))
    TBv = _pad_tb(TBv)
    TV = sum(TBv)

    per_core = []
    for c in range(NCORES):
        idxel, lucel = _side_arrays(*e_sorted[(c, 0)], NBE, TBe[0])
        idxeh, luceh = _side_arrays(*e_sorted[(c, 1)], NBE, TBe[1])
        idxv, lucv = _side_arrays(*v_sorted[c], NBV, TBv)
        rde = np.ones((P, NBE), dtype=np.float32)
        for b in range(NBE):
            n = min(P, EPC - P * b)
            rde[:n, b] = rde_full[c * EPC + P * b: c * EPC + P * b + n]
        rdv = np.ones((P, NBV), dtype=np.float32)
        for b in range(NBV):
            n = min(P, VPC - P * b)
            rdv[:n, b] = rdv_full[c * VPC + P * b: c * VPC + P * b + n]
        per_core.append(dict(
            idxel=_wrap_idx16(idxel), lucel=lucel,
            idxeh=_wrap_idx16(idxeh), luceh=luceh,
            idxv=_wrap_idx16(idxv), lucv=lucv,
            rde=rde, rdv=rdv))
    return dict(TBel=TBe[0], TBeh=TBe[1], TBv=TBv,
                TEl=TEl, TEh=TEh, TV=TV, per_core=per_core)


def _flatten_blocks(TB):
    """[(block, is_first, is_last)] per tile."""
    out = []
    for b, T in enumerate(TB):
        for t in range(T):
            out.append((b, t == 0, t == T - 1))
    return out


def _build(meta, debug=None):
    import concourse.bacc as bacc
    import concourse.bass as bass
    import concourse.mybir as mybir
    import concourse.tile as tile

    f32 = mybir.dt.float32
    bf16 = mybir.dt.bfloat16
    i16 = mybir.dt.int16
    TEl, TEh, TV = meta["TEl"], meta["TEh"], meta["TV"]
    tiles_el = _flatten_blocks(meta["TBel"])
    tiles_eh = _flatten_blocks(meta["TBeh"])
    tiles_v = _flatten_blocks(meta["TBv"])

    nc = bacc.Bacc("TRN2", target_bir_lowering=False, debug=False,
                   num_devices=NCORES)

    xt = nc.dram_tensor("xt", [V, D], bf16, kind="ExternalInput")
    idxel_d = nc.dram_tensor("idxel", [P, TEl * 8], i16, kind="ExternalInput")
    lucel_d = nc.dram_tensor("lucel", [P, TEl], f32, kind="ExternalInput")
    idxeh_d = nc.dram_tensor("idxeh", [P, TEh * 8], i16, kind="ExternalInput")
    luceh_d = nc.dram_tensor("luceh", [P, TEh], f32, kind="ExternalInput")
    idxv_d = nc.dram_tensor("idxv", [P, TV * 8], i16, kind="ExternalInput")
    lucv_d = nc.dram_tensor("lucv", [P, TV], f32, kind="ExternalInput")
    rde_d = nc.dram_tensor("rde", [P, NBE], f32, kind="ExternalInput")
    rdv_d = nc.dram_tensor("rdv", [P, NBV], f32, kind="ExternalInput")
    w_d = [nc.dram_tensor(f"w{i+1}", [D, F_OUT[i]], bf16, kind="ExternalInput")
           for i in range(3)]
    b_d = [nc.dram_tensor(f"b{i+1}x", [P, F_OUT[i]], f32, kind="ExternalInput")
           for i in range(3)]
    iota_d = nc.dram_tensor("iota", [P, P], f32, kind="ExternalInput")
    ident_d = nc.dram_tensor("ident", [P, P], bf16, kind="ExternalInput")
    if debug == "e0":
        out_d = nc.dram_tensor("out", [EPC, D], f32, kind="ExternalOutput")
    elif debug in ("v0", "v1"):
        out_d = nc.dram_tensor("out", [VPC, D], f32, kind="ExternalOutput")
    else:
        out_d = nc.dram_tensor("out", [VPC, F_OUT[2]], f32, kind="ExternalOutput")

    eloc = [nc.dram_tensor(f"eloc{i}", [EPC, D], bf16) for i in range(2)]
    etab = [nc.dram_tensor(f"etab{i}", [E, D], bf16, addr_space="Shared")
            for i in range(2)]
    vloc = [nc.dram_tensor(f"vloc{i}", [VPC, D], bf16) for i in range(2)]
    vtab = [nc.dram_tensor(f"vtab{i}", [V, D], bf16, addr_space="Shared")
            for i in range(2)]
    groups = [list(range(NCORES))]

    with tile.TileContext(nc) as tc:
        with (
            tc.tile_pool(name="const", bufs=1) as cpool,
            tc.tile_pool(name="g", bufs=6) as gpool,
            tc.tile_pool(name="st", bufs=4) as spool,
            tc.tile_pool(name="eo", bufs=3) as eopool,
            tc.tile_pool(name="va", bufs=2) as vapool,
            tc.tile_pool(name="at", bufs=2) as atpool,
            tc.tile_pool(name="ob", bufs=3) as obpool,
            tc.tile_pool(name="sm", bufs=2) as smpool,
            tc.tile_pool(name="ps", bufs=3, space="PSUM") as pspool,
            tc.tile_pool(name="pt", bufs=2, space="PSUM") as ptpool,
            tc.tile_pool(name="po", bufs=2, space="PSUM") as popool,
        ):
            def load_const(dram, shape, tag, dtype=f32):
                t = cpool.tile(shape, dtype, tag=tag)
                nc.sync.dma_start(out=t[:], in_=dram[:])
                return t

            idxel_sb = load_const(idxel_d, [P, TEl * 8], "idxel", i16)
            lucel_sb = load_const(lucel_d, [P, TEl], "lucel")
            idxeh_sb = load_const(idxeh_d, [P, TEh * 8], "idxeh", i16)
            luceh_sb = load_const(luceh_d, [P, TEh], "luceh")
            idxv_sb = load_const(idxv_d, [P, TV * 8], "idxv", i16)
            lucv_sb = load_const(lucv_d, [P, TV], "lucv")
            rde_sb = load_const(rde_d, [P, NBE], "rde")
            rdv_sb = load_const(rdv_d, [P, NBV], "rdv")
            iota_sb = load_const(iota_d, [P, P], "iota")
            ident_sb = load_const(ident_d, [P, P], "ident", bf16)
            w_sb = []
            for i in range(3):
                t0 = cpool.tile([P, F_OUT[i]], bf16, tag=f"w{i}a")
                t1 = cpool.tile([P, F_OUT[i]], bf16, tag=f"w{i}b")
                nc.sync.dma_start(out=t0[:], in_=w_d[i][0:P, :])
                nc.sync.dma_start(out=t1[:], in_=w_d[i][P:2 * P, :])
                w_sb.append((t0, t1))
            b_sb = [load_const(b_d[i], [P, F_OUT[i]], f"bias{i}")
                    for i in range(3)]
            # SBUF accumulator for the two-sweep edge pass
            eacc = cpool.tile([P, NBE * D], f32, tag="eacc")

            def segsum(table_ap, idx_sb, luc_sb, tiles, n_tiles, on_done):
                """Sorted-stream segment sum: gather TC tiles per dma_gather
                chunk, select with is_equal one-hots, accumulate per block
                in PSUM; on_done(block, psum) at each block's last tile."""
                psums = {}
                for ch in range(n_tiles // TC):
                    g = gpool.tile([P, TC * D], bf16, tag="g")
                    nc.gpsimd.dma_gather(
                        g[:].rearrange("p (t d) -> p t d", d=D),
                        table_ap,
                        idx_sb[:, ch * TC * 8:(ch + 1) * TC * 8],
                        TC * P, TC * P, D)
                    st = spool.tile([P, TC * P], bf16, tag="st")
                    nc.vector.tensor_tensor(
                        out=st[:].rearrange("p (t i) -> p t i", i=P),
                        in0=luc_sb[:, ch * TC:(ch + 1) * TC]
                            .unsqueeze(2).to_broadcast([P, TC, P]),
                        in1=iota_sb[:].unsqueeze(1).to_broadcast([P, TC, P]),
                        op=mybir.AluOpType.is_equal,
                    )
                    for j in range(TC):
                        b, first, last = tiles[ch * TC + j]
                        if first:
                            psums[b] = pspool.tile([P, D], f32, tag="ps",
                                                   name=f"ps{b}")
                        nc.tensor.matmul(
                            out=psums[b][:],
                            lhsT=st[:, j * P:(j + 1) * P],
                            rhs=g[:, j * D:(j + 1) * D],
                            start=first, stop=last,
                        )
                        if last:
                            on_done(b, psums.pop(b))

            for layer in range(3):
                table_in = xt if layer == 0 else vtab[(layer + 1) % 2]
                Fo = F_OUT[layer]
                dump_e = debug == "e0" and layer == 0
                dump_v = debug == f"v{layer}"

                # --- edge side: two sweeps (node halves), SBUF-accumulated
                def e_lo(b, ps):
                    nc.vector.tensor_copy(eacc[:, b * D:(b + 1) * D], ps[:])

                def e_hi(b, ps, layer=layer, dump_e=dump_e):
                    esb = eopool.tile([P, D], bf16, tag="eo")
                    nc.vector.tensor_add(out=eacc[:, b * D:(b + 1) * D],
                                         in0=eacc[:, b * D:(b + 1) * D],
                                         in1=ps[:])
                    nc.vector.tensor_scalar_mul(
                        esb[:], eacc[:, b * D:(b + 1) * D], rde_sb[:, b:b + 1])
                    cnt = min(P, EPC - P * b)
                    if dump_e:
                        esf = eopool.tile([P, D], f32, tag="eof")
                        nc.vector.tensor_scalar_mul(
                            esf[:], eacc[:, b * D:(b + 1) * D],
                            rde_sb[:, b:b + 1])
                        nc.sync.dma_start(out=out_d[P * b:P * b + cnt, :],
                                          in_=esf[:cnt, :])
                    else:
                        nc.sync.dma_start(
                            out=eloc[layer % 2][P * b:P * b + cnt, :],
                            in_=esb[:cnt, :])

                segsum(xt[0:VHALF, :] if layer == 0
                       else vtab[(layer + 1) % 2][0:VHALF, :],
                       idxel_sb, lucel_sb, tiles_el, TEl, e_lo)
                segsum(xt[VHALF:V, :] if layer == 0
                       else vtab[(layer + 1) % 2][VHALF:V, :],
                       idxeh_sb, luceh_sb, tiles_eh, TEh, e_hi)
                if dump_e:
                    break
                nc.gpsimd.collective_compute(
                    "AllGather", mybir.AluOpType.bypass, replica_groups=groups,
                    ins=[eloc[layer % 2][:].opt()], outs=[etab[layer % 2][:].opt()],
                )

                def v_done(b, ps, layer=layer, Fo=Fo, dump_v=dump_v):
                    asb = vapool.tile([P, D], bf16, tag="va")
                    nc.vector.tensor_scalar_mul(asb[:], ps[:], rdv_sb[:, b:b + 1])
                    ptp = ptpool.tile([P, D], bf16, tag="pt")
                    nc.tensor.transpose(out=ptp[:, 0:P], in_=asb[:, 0:P],
                                        identity=ident_sb[:])
                    nc.tensor.transpose(out=ptp[:, P:D], in_=asb[:, P:D],
                                        identity=ident_sb[:])
                    att = atpool.tile([P, D], bf16, tag="at")
                    nc.vector.tensor_copy(att[:], ptp[:])
                    pop = popool.tile([P, Fo], f32, tag="po")
                    nc.tensor.matmul(out=pop[:], lhsT=att[:, 0:P],
                                     rhs=w_sb[layer][0][:], start=True, stop=False)
                    nc.tensor.matmul(out=pop[:], lhsT=att[:, P:D],
                                     rhs=w_sb[layer][1][:], start=False, stop=True)
                    cnt = min(P, VPC - P * b)
                    if layer < 2:
                        osb = obpool.tile([P, Fo], bf16, tag="ob")
                        nc.vector.tensor_add(out=osb[:], in0=pop[:],
                                             in1=b_sb[layer][:])
                        nc.scalar.activation(out=osb[:], in_=osb[:],
                                             func=mybir.ActivationFunctionType.Relu)
                        if dump_v:
                            osf = obpool.tile([P, Fo], f32, tag="obf")
                            nc.vector.tensor_copy(osf[:], osb[:])
                            nc.sync.dma_start(out=out_d[P * b:P * b + cnt, :],
                                              in_=osf[:cnt, :])
                        else:
                            nc.sync.dma_start(
                                out=vloc[layer % 2][P * b:P * b + cnt, :],
                                in_=osb[:cnt, :])
                    else:
                        osb = obpool.tile([P, Fo], f32, tag="obf32")
                        nc.vector.tensor_add(out=osb[:], in0=pop[:],
                                             in1=b_sb[layer][:])
                        negmax = smpool.tile([P, 1], f32, tag="negmax")
                        nc.vector.tensor_reduce(
                            out=negmax[:], in_=osb[:], axis=mybir.AxisListType.X,
                            op=mybir.AluOpType.max, negate=True)
                        expt = smpool.tile([P, Fo], f32, tag="expt")
                        sumexp = smpool.tile([P, 1], f32, tag="sumexp")
                        nc.scalar.activation(
                            out=expt[:], in_=osb[:],
                            func=mybir.ActivationFunctionType.Exp,
                            bias=negmax[:, 0:1], accum_out=sumexp[:, 0:1])
                        logsum = smpool.tile([P, 1], f32, tag="logsum")
                        nc.scalar.activation(
                            out=logsum[:], in_=sumexp[:],
                            func=mybir.ActivationFunctionType.Ln)
                        shift = smpool.tile([P, 1], f32, tag="shift")
                        nc.vector.tensor_sub(out=shift[:], in0=negmax[:],
                                             in1=logsum[:])
                        res = smpool.tile([P, Fo], f32, tag="res")
                        nc.vector.tensor_scalar_add(res[:], osb[:], shift[:, 0:1])
                        nc.sync.dma_start(out=out_d[P * b:P * b + cnt, :],
                                          in_=res[:cnt, :])

                segsum(etab[layer % 2][:], idxv_sb, lucv_sb, tiles_v, TV, v_done)
                if dump_v:
                    break
                if layer < 2:
                    nc.gpsimd.collective_compute(
                        "AllGather", mybir.AluOpType.bypass,
                        replica_groups=groups,
                        ins=[vloc[layer % 2][:].opt()],
                        outs=[vtab[layer % 2][:].opt()],
                    )
    nc.finalize()
    return nc


_CACHE = {}


def make_in_maps(meta, X, W1, b1, W2, b2, W3, b3):
    iota = np.broadcast_to(np.arange(P, dtype=np.float32), (P, P)).copy()
    ident = np.eye(P, dtype=np.float32).astype(ml_dtypes.bfloat16)
    Xb = np.ascontiguousarray(np.asarray(X, dtype=np.float32)).astype(
        ml_dtypes.bfloat16)
    ws = [np.ascontiguousarray(np.asarray(w, dtype=np.float32)).astype(
        ml_dtypes.bfloat16) for w in (W1, W2, W3)]
    bs = [np.broadcast_to(np.asarray(b, dtype=np.float32), (P, len(b))).copy()
          for b in (b1, b2, b3)]
    in_maps = []
    for c in range(NCORES):
        pc = meta["per_core"][c]
        in_maps.append({
            "xt": Xb,
            "idxel": pc["idxel"], "lucel": pc["lucel"],
            "idxeh": pc["idxeh"], "luceh": pc["luceh"],
            "idxv": pc["idxv"], "lucv": pc["lucv"],
            "rde": pc["rde"], "rdv": pc["rdv"],
            "w1": ws[0], "w2": ws[1], "w3": ws[2],
            "b1x": bs[0], "b2x": bs[1], "b3x": bs[2],
            "iota": iota, "ident": ident,
        })
    return in_maps


def kernel(X, node_idx, edge_idx, W1, b1, W2, b2, W3, b3):
    from concourse import bass_utils

    ni = np.asarray(node_idx, dtype=np.int32)
    ei = np.asarray(edge_idx, dtype=np.int32)

    key = hashlib.sha1(ni.tobytes() + ei.tobytes()).hexdigest()
    if key not in _CACHE:
        meta = _preprocess(ni, ei)
        nc = _build(meta)
        _CACHE[key] = (meta, nc)
    meta, nc = _CACHE[key]

    in_maps = make_in_maps(meta, X, W1, b1, W2, b2, W3, b3)
    res = bass_utils.run_bass_kernel_spmd(nc, in_maps, list(range(NCORES)))
    return np.concatenate([res.results[c]["out"] for c in range(NCORES)], axis=0)
